# revision 2
# baseline (speedup 1.0000x reference)
"""Log-domain Sinkhorn (B=16, N=M=2048, eps=0.05, 50 iters) on 8 trn2 cores.

Data-parallel over batch: each of the 8 cores runs 2 independent [N,M]
Sinkhorn problems entirely on-chip.

Device kernel:
- Marginals are folded into the kernel matrices once at setup:
      ekar[i,j] = (r_i+1e-12) * exp(-cost_ij/eps)   ([i-part, j-free] layout)
      ekbc[j,i] = (c_j+1e-12) * exp(-cost_ij/eps)   ([j-part, i-free] layout)
  (via the activation bias: e^(-c/eps + ln m) = m * e^(-c/eps)), so each
  half-iteration is p = 1/(ekbc-stream q), q = 1/(ekar-stream p) and the
  per-iteration glue is a single DVE reciprocal per 512-quarter, straight
  out of PSUM.
- Both EK layouts are built directly from the (f16) cost input — the
  transposed layout via DMA-transpose from DRAM — and stay resident in SBUF
  (2 x 8 MiB bf16); iterations touch no HBM.
- Matvec halves run on the PE with the dual vector as the stationary operand
  ([128,1] per chunk, ~1-cycle weight load) and EK as the moving operand,
  4-way column-tiled (tile_position) so four 512-wide output quarters
  stream concurrently through separate XBUSes.
- The free-major -> partition-major transpose of the new dual vector is 16
  K=1 one-column matmuls against ones (PE), overlapped with the DVE glue.
- Finale computes T = p * ekar * ((c+eps) q) from SBUF-resident ekar (one
  fused DVE op per 128-row chunk; cost is never re-read) and writes f16.

Host path:
- cost ships as f16 (threaded f32->f16 convert), halving H2D bytes.
- Inputs go per-device via device_put + make_array_from_single_device_arrays
  (no host-side concatenation of the 256 MB batch).
- The donated output buffers are created on-device by a jitted zeros fn
  (nothing shipped over the wire for them).
- Output comes back f16 and is upcast to f32 into the result array in
  worker threads, per shard, while later shards are still in flight.
"""
import sys

sys.path.insert(0, "/opt/trn_rl_repo")

import numpy as np
from concurrent.futures import ThreadPoolExecutor
from contextlib import ExitStack

import concourse.bass as bass  # noqa: F401  (engine namespaces used via nc)
import concourse.tile as tile
from concourse import bacc, mybir

EPS = 0.05
ITERS = 50
N = 2048
P = 128
NCH = N // P   # 16 chunks of 128
NQ = 4         # column-tile quarters
QW = N // NQ   # 512
BPC = 2        # batches per core
NCORES = 8
REPS = 1       # outer For_i repeats of everything (timing experiments only)

F32 = mybir.dt.float32
BF16 = mybir.dt.bfloat16
F16 = mybir.dt.float16
AF = mybir.ActivationFunctionType
ALU = mybir.AluOpType

COST_DT = F16   # wire/HBM dtype of cost
OUT_DT = F16    # wire/HBM dtype of output


def _sinkhorn_kernel(tc, out_ap, cost_ap, src_ap, tgt_ap):
    nc = tc.nc
    with ExitStack() as ctx:
        consts = ctx.enter_context(tc.tile_pool(name="consts", bufs=1))
        ekp = ctx.enter_context(tc.tile_pool(name="ek", bufs=1))
        vec = ctx.enter_context(tc.tile_pool(name="vec", bufs=1))
        stage = ctx.enter_context(tc.tile_pool(name="stage", bufs=3))
        ostage = ctx.enter_context(tc.tile_pool(name="ostage", bufs=3))
        psum = ctx.enter_context(tc.tile_pool(name="psum", bufs=1, space="PSUM"))

        ones_bf = consts.tile([P, P], BF16)
        nc.vector.memset(ones_bf, 1.0)

        ekar = ekp.tile([P, NCH, N], BF16, tag="ekar")  # [i', ic, j]
        ekbc = ekp.tile([P, NCH, N], BF16, tag="ekbc")  # [j', jc, i]

        # partition-major marginals and dual vectors
        r_pm = vec.tile([P, NCH], F32, tag="r_pm")      # (r+1e-12)[128ic+p]
        c_pm = vec.tile([P, NCH], F32, tag="c_pm")      # (c+1e-12)[128jc+p]
        lnr_pm = vec.tile([P, NCH], F32, tag="lnr_pm")  # ln(r+1e-12)
        lnc_pm = vec.tile([P, NCH], F32, tag="lnc_pm")  # ln(c+1e-12)
        qv_pm = vec.tile([P, NCH], BF16, tag="qv_pm")   # q = ev/(c+eps)
        pu_pm = vec.tile([P, NCH], BF16, tag="pu_pm")   # p = eu/(r+eps)
        pu_pm_f = vec.tile([P, NCH], F32, tag="pu_pm_f")
        q0_f = vec.tile([P, NCH], F32, tag="q0_f")
        # free-major rows: quarter q lives on partition 32q
        c_sb = vec.tile([P, QW], F32, tag="c_sb")       # raw c rows (finale)
        pu_row = vec.tile([P, QW], BF16, tag="pu_row")
        qv_row = vec.tile([P, QW], BF16, tag="qv_row")
        ev_row = vec.tile([P, QW], BF16, tag="ev_row")  # finale (c+eps)*q rows
        evb = vec.tile([P, N], BF16, tag="evb")         # finale broadcast of ev

        pmv = [
            psum.tile([P, QW], F32, tag=f"mv{q}", name=f"mv{q}") for q in range(NQ)
        ]
        ppm_u = psum.tile([P, NCH], F32, tag="ppm_u")
        ppm_v = psum.tile([P, NCH], F32, tag="ppm_v")

        def matvec(ek, vin_pm, out_rows, ppm, vout_pm):
            """One half-iteration: s_q = sum_c ek[:, c, q-slice]^T @ vin_pm[:, c]
            (col-tiled), out_rows = 1/s (DVE recip from PSUM), then transpose
            rows into ppm via 16 K=1 matmuls and round to bf16 vout_pm."""
            for c in range(NCH):
                for q in range(NQ):
                    nc.tensor.matmul(
                        pmv[q][32 * q:32 * q + 1, :],
                        vin_pm[:, c:c + 1],
                        ek[:, c, QW * q:QW * (q + 1)],
                        start=(c == 0),
                        stop=(c == NCH - 1),
                        tile_position=(0, 32 * q),
                    )
            for q in range(NQ):
                with nc.allow_low_precision(reason="bf16 sinkhorn duals"):
                    nc.vector.reciprocal(
                        out_rows[32 * q:32 * q + 1, :], pmv[q][32 * q:32 * q + 1, :]
                    )
                for k in range(NQ):
                    c = NQ * q + k
                    nc.tensor.matmul(
                        ppm[:, c:c + 1],
                        out_rows[32 * q:32 * q + 1, P * k:P * (k + 1)],
                        ones_bf[32 * q:32 * q + 1, 0:1],
                        start=True,
                        stop=True,
                        tile_position=(32 * q, 0),
                    )
            nc.vector.tensor_copy(vout_pm, ppm)

        def whole_body():
          for b in range(BPC):
            # ---- marginals ----
            rv = src_ap[b].rearrange("(cc p) -> p cc", p=P)
            cv = tgt_ap[b].rearrange("(cc p) -> p cc", p=P)
            nc.sync.dma_start(out=r_pm, in_=rv)
            nc.sync.dma_start(out=c_pm, in_=cv)
            nc.vector.tensor_scalar_add(r_pm, r_pm, 1e-12)
            nc.vector.tensor_scalar_add(c_pm, c_pm, 1e-12)
            for q in range(NQ):
                nc.sync.dma_start(
                    out=c_sb[32 * q:32 * q + 1, :], in_=tgt_ap[b, QW * q:QW * (q + 1)]
                )
            # q0 = 1/(c+eps)
            nc.vector.reciprocal(q0_f, c_pm)
            nc.vector.tensor_copy(qv_pm, q0_f)
            nc.scalar.activation(lnr_pm, r_pm, AF.Ln)
            nc.scalar.activation(lnc_pm, c_pm, AF.Ln)

            # ---- EK in both layouts straight from (f16) cost; marginals
            # folded in via the exp bias: e^(-c/eps + ln m) = m * e^(-c/eps)
            for ic in range(NCH):
                ct = stage.tile([P, N], COST_DT)
                nc.sync.dma_start(out=ct, in_=cost_ap[b, ic * P:(ic + 1) * P, :])
                nc.scalar.activation(
                    ekar[:, ic, :], ct, AF.Exp,
                    scale=-1.0 / EPS, bias=lnr_pm[:, ic:ic + 1],
                )
            for jc in range(NCH):
                ctt = stage.tile([P, N], COST_DT, tag="ctt")
                nc.sync.dma_start_transpose(
                    out=ctt, in_=cost_ap[b, :, jc * P:(jc + 1) * P]
                )
                nc.scalar.activation(
                    ekbc[:, jc, :], ctt, AF.Exp,
                    scale=-1.0 / EPS, bias=lnc_pm[:, jc:jc + 1],
                )

            # ---- ITERS Sinkhorn iterations, all on-chip ----
            for _ in range(ITERS):
                matvec(ekbc, qv_pm, pu_row, ppm_u, pu_pm)
                matvec(ekar, pu_pm, qv_row, ppm_v, qv_pm)

            # ---- finale: T = p * ekar * ev,  ev = (c+eps)*q ----
            nc.vector.tensor_copy(pu_pm_f, ppm_u)
            for q in range(NQ):
                nc.vector.scalar_tensor_tensor(
                    out=ev_row[32 * q:32 * q + 1, :],
                    in0=c_sb[32 * q:32 * q + 1, :],
                    scalar=1e-12,
                    in1=qv_row[32 * q:32 * q + 1, :],
                    op0=ALU.add,
                    op1=ALU.mult,
                )
                nc.tensor.matmul(
                    pmv[q],
                    ones_bf[32 * q:32 * q + 1, :],
                    ev_row[32 * q:32 * q + 1, :],
                    start=True,
                    stop=True,
                    tile_position=(32 * q, 0),
                )
                nc.vector.tensor_copy(evb[:, QW * q:QW * (q + 1)], pmv[q])
            for ic in range(NCH):
                ot = ostage.tile([P, N], OUT_DT)
                nc.vector.scalar_tensor_tensor(
                    out=ot,
                    in0=ekar[:, ic, :],
                    scalar=pu_pm_f[:, ic:ic + 1],
                    in1=evb,
                    op0=ALU.mult,
                    op1=ALU.mult,
                )
                nc.sync.dma_start(out=out_ap[b, ic * P:(ic + 1) * P, :], in_=ot)

        if REPS == 1:
            whole_body()
        else:
            with tc.For_i(0, REPS, 1):
                whole_body()


_CACHE = {}
_POOL = ThreadPoolExecutor(16)


def _get_compiled():
    if "nc" not in _CACHE:
        nc = bacc.Bacc(
            "TRN2", target_bir_lowering=False, debug=False, num_devices=NCORES
        )
        cost = nc.dram_tensor("cost", [BPC, N, N], COST_DT, kind="ExternalInput").ap()
        src = nc.dram_tensor("src", [BPC, N], F32, kind="ExternalInput").ap()
        tgt = nc.dram_tensor("tgt", [BPC, N], F32, kind="ExternalInput").ap()
        out = nc.dram_tensor("out", [BPC, N, N], OUT_DT, kind="ExternalOutput").ap()
        with tile.TileContext(nc) as tc:
            _sinkhorn_kernel(tc, out, cost, src, tgt)
        nc.compile()
        _CACHE["nc"] = nc
    return _CACHE["nc"]


# ---------------------------------------------------------------------------
# Host execution path (custom PJRT): per-device sharded inputs (no host-side
# concat), donated output buffers created on-device, threaded dtype
# conversions, async per-shard D2H.
# ---------------------------------------------------------------------------

def _build_runner(nc):
    import jax
    import jax.numpy as jnp
    from jax.sharding import Mesh, PartitionSpec, NamedSharding
    from concourse import bass2jax

    bass2jax.install_neuronx_cc_hook()
    try:
        from jax.experimental.shard_map import shard_map
    except ImportError:
        from jax import shard_map  # newer jax

    partition_name = nc.partition_id_tensor.name if nc.partition_id_tensor else None
    in_names, out_names, out_avals = [], [], []
    for alloc in nc.m.functions[0].allocations:
        if not isinstance(alloc, mybir.MemoryLocationSet):
            continue
        name = alloc.memorylocations[0].name
        if alloc.kind == "ExternalInput":
            if name != partition_name:
                in_names.append(name)
        elif alloc.kind == "ExternalOutput":
            out_names.append(name)
            shape = tuple(alloc.tensor_shape)
            dtype = mybir.dt.np(alloc.dtype)
            out_avals.append(jax.core.ShapedArray(shape, dtype))
    n_params = len(in_names)
    n_outs = len(out_avals)
    all_in_names = list(in_names) + out_names
    if partition_name is not None:
        all_in_names.append(partition_name)
    donate = tuple(range(n_params, n_params + n_outs))

    def _body(*args):
        operands = list(args)
        if partition_name is not None:
            operands.append(bass2jax.partition_id_tensor())
        outs = bass2jax._bass_exec_p.bind(
            *operands,
            out_avals=tuple(out_avals),
            in_names=tuple(all_in_names),
            out_names=tuple(out_names),
            lowering_input_output_aliases=(),
            sim_require_finite=True,
            sim_require_nnan=True,
            nc=nc,
        )
        return tuple(outs)

    devices = jax.devices()[:NCORES]
    mesh = Mesh(np.asarray(devices), ("core",))
    spec = PartitionSpec("core")
    ns = NamedSharding(mesh, spec)
    sharded = jax.jit(
        shard_map(
            _body,
            mesh=mesh,
            in_specs=(spec,) * (n_params + n_outs),
            out_specs=(spec,) * n_outs,
            check_rep=False,
        ),
        donate_argnums=donate,
        keep_unused=True,
    )
    zshapes = [(NCORES * a.shape[0], *a.shape[1:]) for a in out_avals]
    zdtypes = [a.dtype for a in out_avals]
    zfn = jax.jit(
        lambda: tuple(jnp.zeros(s, d) for s, d in zip(zshapes, zdtypes)),
        out_shardings=tuple(ns for _ in out_avals),
    )
    return {
        "jax": jax,
        "sharded": sharded,
        "zfn": zfn,
        "ns": ns,
        "devices": devices,
        "in_names": in_names,
    }


def _put_sharded(runner, shards):
    jax = runner["jax"]
    s0 = shards[0]
    gshape = (len(shards) * s0.shape[0], *s0.shape[1:])
    dbs = [jax.device_put(shards[k], runner["devices"][k]) for k in range(len(shards))]
    return jax.make_array_from_single_device_arrays(gshape, runner["ns"], dbs)


def kernel(cost, source_marginal, target_marginal):
    cost = np.asarray(cost)
    src = np.ascontiguousarray(source_marginal, dtype=np.float32)
    tgt = np.ascontiguousarray(target_marginal, dtype=np.float32)
    B = cost.shape[0]
    assert B == BPC * NCORES
    if "runner" not in _CACHE:
        _CACHE["runner"] = _build_runner(_get_compiled())
    runner = _CACHE["runner"]

    cost_shards = list(
        _POOL.map(
            lambda k: cost[k * BPC:(k + 1) * BPC].astype(np.float16), range(NCORES)
        )
    )
    shard_maps = {
        "cost": cost_shards,
        "src": [src[k * BPC:(k + 1) * BPC] for k in range(NCORES)],
        "tgt": [tgt[k * BPC:(k + 1) * BPC] for k in range(NCORES)],
    }
    gin = [_put_sharded(runner, shard_maps[name]) for name in runner["in_names"]]
    zeros = runner["zfn"]()
    outs = runner["sharded"](*gin, *zeros)
    out = outs[0]

    result = np.empty((B, N, N), dtype=np.float32)
    shards = sorted(out.addressable_shards, key=lambda s: s.index[0].start or 0)
    for sh in shards:
        try:
            sh.data.copy_to_host_async()
        except Exception:
            pass
    futs = []
    for sh in shards:
        start = sh.index[0].start or 0
        data = np.asarray(sh.data)  # blocks on this shard's D2H
        futs.append(
            _POOL.submit(
                lambda s, d: result.__setitem__(
                    slice(s, s + d.shape[0]), d.astype(np.float32)
                ),
                start,
                data,
            )
        )
    for f in futs:
        f.result()
    return result
